# revision 26
# baseline (speedup 1.0000x reference)
"""Causal multi-head attention block on 8 trn2 NeuronCores.

Problem: B=2, S=2048, D=768, H=12, Dh=64 (fp32), causal softmax attention
with QKV projections and output projection summed over heads.

Sharding: tensor-parallel over heads x data-parallel over batch.
core c in [0,8): b = c//4, heads = {3g, 3g+1, 3g+2} with g = c%4.
Each core computes the partial output sum over its 3 heads for its batch;
the host sums the 4 partials per batch (the TP all-reduce) and stacks.

v5 layout (all device I/O in bf16, host pre-casts/pre-packs):
  - x^T [768, 2048] bf16 loaded directly to SBUF (no on-device cast).
  - QKV projections per sq-column j are emitted interleaved with the
    attention loops of column j-1 so ScalarE exp work starts early.
  - Pair heads (h0,h1) stacked on PE row halves -> concurrent K=64 score
    matmuls; solo head h2 duplicated on both partition halves and two
    sk-tiles processed per step on alternating halves (also concurrent),
    with a single merged exp per step.
  - z matmuls carry a ones column for the softmax denominator; normalize
    uses reciprocal straight from PSUM + row broadcast + one fp32 multiply.
  - Output projection accumulates the head pair (K=128) and solo (K=64)
    into shared PSUM, copied to bf16 and stored as one DMA per row block.

Biases are all zeros per the problem spec (fill=zeros); b_O is applied on
the host if nonzero (exact). b_Q/b_K/b_V are asserted zero.
"""

import os
import sys
import types
import numpy as np

B, S, D, H, DH = 2, 2048, 768, 12, 64
N_CORES = 8
P = 128
NK = D // P      # 6 contraction chunks
NJ = S // 512    # 4 sq tiles of 512
NI = S // P      # 16 sk tiles of 128
SQT = 512

# SBUF->SBUF broadcast DMA for the denominator row; if the toolchain
# rejects it we fall back to a DRAM bounce.
SBUF_BCAST = os.environ.get("BASS_SBUF_BCAST", "0") == "1"

_PROGRAM = None
LAST_RESULTS = None


def _install_ntff_shim():
    """antenv.axon_hooks is missing in this image; shim it so trace=True works."""
    if "antenv.axon_hooks" in sys.modules:
        return
    try:
        from trn_agent_boot.trn_boot import _ntff_profile_via_ctypes
        m = types.ModuleType("antenv.axon_hooks")
        hook = _ntff_profile_via_ctypes("/opt/axon/libaxon_pjrt.so")
        m.get_axon_ntff_profile_hook = lambda: hook
        m.set_axon_ntff_profile_hook = lambda h: None
        sys.modules["antenv.axon_hooks"] = m
    except Exception:
        pass


def _build_program():
    import concourse.bass as bass
    import concourse.mybir as mybir
    from concourse import bacc
    from concourse.tile import TileContext
    from concourse.bass import ts, ds

    fp32 = mybir.dt.float32
    bf16 = mybir.dt.bfloat16
    Exp = mybir.ActivationFunctionType.Exp
    Mult = mybir.AluOpType.mult

    nc = bacc.Bacc("TRN2", target_bir_lowering=False, debug=False,
                   num_devices=N_CORES)

    xT = nc.dram_tensor("xT", (D, S), bf16, kind="ExternalInput")
    wq2 = nc.dram_tensor("wq2", (P, NK, 128), bf16, kind="ExternalInput")
    wk2 = nc.dram_tensor("wk2", (P, NK, 128), bf16, kind="ExternalInput")
    wqk3 = nc.dram_tensor("wqk3", (P, NK, 128), bf16, kind="ExternalInput")
    wvb_src = nc.dram_tensor("wv", (P, NK, 192), bf16, kind="ExternalInput")
    wo2 = nc.dram_tensor("wo2", (128, D), bf16, kind="ExternalInput")
    wo3 = nc.dram_tensor("wo3", (DH, D), bf16, kind="ExternalInput")
    maskin = nc.dram_tensor("mask", (P, P), bf16, kind="ExternalInput")
    out = nc.dram_tensor("out", (S, D), bf16, kind="ExternalOutput")

    with TileContext(nc) as tc:
        with tc.tile_pool(name="work", bufs=1) as work, \
             tc.tile_pool(name="epool", bufs=4) as epool, \
             tc.tile_pool(name="zsb", bufs=2) as zsb, \
             tc.tile_pool(name="zcol", bufs=2) as zcol, \
             tc.tile_pool(name="dram", bufs=2, space="DRAM") as dram, \
             tc.tile_pool(name="psum", bufs=2, space="PSUM") as psum:

            # ---------------- persistent SBUF tiles ----------------
            QT2 = work.tile([P, S], bf16, name="QT2")   # h0 rows 0:64, h1 rows 64:128
            KT2 = work.tile([P, S], bf16, name="KT2")
            QT3d = work.tile([P, S], bf16, name="QT3d")  # h2 Q^T duplicated both halves
            KT3d = work.tile([P, S], bf16, name="KT3d")  # h2 K^T duplicated both halves
            V_all = work.tile([P, NI, 3, 65], bf16, name="V_all")
            xTb = work.tile([P, NK, S], bf16, name="xTb")
            wq2b = work.tile([P, NK, 128], bf16, name="wq2b")
            wk2b = work.tile([P, NK, 128], bf16, name="wk2b")
            wqk3b = work.tile([P, NK, 128], bf16, name="wqk3b")
            wvb = work.tile([P, NK, 192], bf16, name="wvb")
            wo2b = work.tile([P, D], bf16, name="wo2b")
            wo3b = work.tile([64, D], bf16, name="wo3b")
            maskb = work.tile([P, P], bf16, name="maskb")
            warm = work.tile([1, 8], fp32, name="warm")

            # ---------------- input DMAs (no casts needed) ----------------
            # d-major full-row x chunks (fat descriptors), split across both
            # HWDGE rings (sync + scalar); weights needed first lead each ring.
            nc.sync.dma_start(wq2b[:], wq2[:])
            for k in range(NK):
                eng = nc.sync if (k % 2 == 0) else nc.scalar
                eng.dma_start(xTb[:, k, 0:1024], xT[ts(k, P), 0:1024])
            nc.sync.dma_start(wk2b[:], wk2[:])
            nc.sync.dma_start(wqk3b[:], wqk3[:])
            nc.scalar.dma_start(wvb[:], wvb_src[:])
            for k in range(NK):
                eng = nc.sync if (k % 2 == 0) else nc.scalar
                eng.dma_start(xTb[:, k, 1024:2048], xT[ts(k, P), 1024:2048])
            nc.scalar.dma_start(wo2b[:], wo2[:])
            nc.scalar.dma_start(wo3b[:], wo3[:])
            nc.scalar.dma_start(maskb[:], maskin[:])

            # Pre-load the exp activation table while DMAs land.
            nc.vector.memset(warm[:], 0.0)
            nc.scalar.activation(warm[:], warm[:], Exp)
            nc.vector.memset(V_all[:, :, :, 64], 1.0)

            # ---------------- phase emitters ----------------

            wpfill = psum.tile([64, 64], fp32, tag="o", name="wpfill",
                               bufs=1)

            wpf2 = psum.tile([64, SQT], fp32, tag="o", name="wpf2", bufs=1)

            def fill_mm(n):
                for _ in range(n):
                    nc.tensor.matmul(wpfill[:], wdum[:, 0:64], wdum[:, 64:128],
                                     start=True, stop=True,
                                     skip_group_check=True)

            def fill_big(n):
                for _ in range(n):
                    nc.tensor.matmul(wpf2[:], wdum[:, 0:64], wdumw[:],
                                     start=True, stop=True,
                                     skip_group_check=True)

            def qk_group(st_i, which, tag, fills=0):
                sl = ts(st_i, SQT)
                pt = psum.tile([P, SQT], fp32, tag=tag, name=f"qkp{which}",
                               bufs=(2 if tag == "s" else 1))
                wsel = (wq2b, wk2b, wqk3b)[which]
                for k in range(NK):
                    if fills:
                        fill_mm(fills)
                    nc.tensor.matmul(pt[:], wsel[:, k, :], xTb[:, k, sl],
                                     start=(k == 0), stop=(k == NK - 1))
                if which == 0:
                    nc.vector.tensor_copy(QT2[:, sl], pt[:])
                elif which == 1:
                    nc.vector.tensor_copy(KT2[:, sl], pt[:])
                else:
                    nc.vector.tensor_copy(QT3d[0:64, sl], pt[0:64, :])
                    nc.vector.tensor_copy(KT3d[64:128, sl], pt[64:128, :])
                    # duplicate to the other half for row-tiled matmuls
                    nc.sync.dma_start(QT3d[64:128, sl], QT3d[0:64, sl])
                    nc.sync.dma_start(KT3d[0:64, sl], KT3d[64:128, sl])

            def v_group(t, tag):
                vp = psum.tile([P, SQT], fp32, tag=tag, name="vp",
                               bufs=(2 if tag == "s" else 1))
                for k in range(NK):
                    nc.tensor.matmul(vp[:, 0:192], xTb[:, k, ts(t, P)],
                                     wvb[:, k, :],
                                     start=(k == 0), stop=(k == NK - 1))
                nc.vector.tensor_copy(
                    V_all[:, t, :, 0:64],
                    vp[:, 0:192].rearrange("p (h e) -> p h e", h=3))

            wdum = work.tile([64, P], bf16, name="wdum")
            wdumw = work.tile([64, SQT], bf16, name="wdumw")
            ones64 = work.tile([1, 64], bf16, name="ones64")
            nc.vector.memset(wdum[:], 0.0)
            nc.vector.memset(wdumw[:], 0.0)
            nc.vector.memset(ones64[:], 1.0)
            qk_group(0, 0, "s", fills=6)
            qk_group(0, 1, "s")
            qk_group(0, 2, "s")

            # ---------------- attention ----------------

            prev_work = []  # deferred out-proj closures from the previous column

            def pop_prev():
                if prev_work:
                    prev_work.pop(0)()

            for j in range(NJ):
                zT2 = zcol.tile([P, SQT], bf16, tag="zT2", name="zT2")
                zT3 = zcol.tile([64, SQT], bf16, tag="zT3", name="zT3")
                n_i = 4 * j + 4
                last = (j == NJ - 1)

                def col0_of(i):
                    return P * (i - 4 * j) if i >= 4 * j else 0

                z_pss = [psum.tile([P, SQT], fp32, tag="z", name=f"z_ps{h}",
                                   bufs=3) for h in range(3)]

                def norm_a(z_ps, bcast="dram"):
                    # tensor_copy handles the partition-64 -> 0 shift; the
                    # custom-DVE reciprocal does not (reads the wrong lane).
                    drow = zsb.tile([1, SQT], fp32, tag="drow", name="drow",
                                    bufs=3)
                    nc.vector.tensor_copy(drow[:], z_ps[64:65, :])
                    rsb = zsb.tile([1, SQT], fp32, tag="rsb", name="rsb", bufs=3)
                    nc.vector.reciprocal_approx_fast(rsb[:], drow[:])
                    rbc = zsb.tile([64, SQT], fp32, tag="rbc", name="rbc", bufs=3)
                    if bcast == "mm":
                        rs16 = zsb.tile([1, SQT], bf16, tag="rs16", name="rs16",
                                        bufs=2)
                        nc.vector.tensor_copy(rs16[:], rsb[:])
                        obp = psum.tile([64, SQT], fp32, tag="o", name="obp",
                                        bufs=1)
                        nc.tensor.matmul(obp[:], ones64[:], rs16[:],
                                         start=True, stop=True,
                                         skip_group_check=True)
                        nc.vector.tensor_copy(rbc[:], obp[:])
                    else:
                        rscr = dram.tile([1, SQT], fp32, name="rscr")
                        nc.sync.dma_start(rscr[:], rsb[:])
                        nc.sync.dma_start(rbc[:], rscr[:].to_broadcast((64, SQT)))
                    return rbc

                def norm_b(h, z_ps, rbc, zT2=zT2, zT3=zT3):
                    if h == 0:
                        nc.vector.tensor_tensor(zT2[0:64, :], z_ps[0:64, :],
                                                rbc[:], Mult)
                    elif h == 1:
                        z1t = zcol.tile([64, SQT], bf16, tag="z1t", name="z1t")
                        nc.vector.tensor_tensor(z1t[:], z_ps[0:64, :],
                                                rbc[:], Mult)
                        nc.sync.dma_start(zT2[64:128, :], z1t[:])
                    else:
                        nc.vector.tensor_tensor(zT3[:], z_ps[0:64, :],
                                                rbc[:], Mult)

                # ---- pair i-loop ----
                def s_pair_start(i):
                    col0 = col0_of(i)
                    s_ps = psum.tile([P, 2, SQT], fp32, tag="s", name="s_ps",
                                     bufs=2)
                    for h in range(2):
                        nc.tensor.matmul(
                            s_ps[:, h, col0:SQT],
                            KT2[64 * h:64 * h + 64, ts(i, P)],
                            QT2[64 * h:64 * h + 64, ds(SQT * j + col0,
                                                       SQT - col0)],
                            start=True, stop=True)
                    return s_ps, col0

                pend = s_pair_start(0)
                if j == 0:
                    # emitted after the first score matmuls so the first exp
                    # issues as early as possible
                    for t in range(4):
                        v_group(t, "s")
                for i in range(n_i):
                    s_ps, col0 = pend
                    E_t = epool.tile([P, 2, SQT], bf16, tag="ep", name="E_t",
                                     bufs=4)
                    nc.scalar.activation(E_t[:, :, col0:SQT],
                                         s_ps[:, :, col0:SQT], Exp, scale=0.125)
                    if i >= 4 * j:
                        nc.vector.tensor_tensor(
                            E_t[:, :, col0:col0 + P], E_t[:, :, col0:col0 + P],
                            maskb[:, None, :].to_broadcast((P, 2, P)), Mult)
                    if i + 1 < n_i:
                        pend = s_pair_start(i + 1)
                    pop_prev()
                    for h in range(2):
                        nc.tensor.matmul(
                            z_pss[h][0:65, col0:SQT],
                            V_all[:, i, h, :],
                            E_t[:, h, col0:SQT],
                            start=(i == 0), stop=(i == n_i - 1),
                            skip_group_check=True)
                    pop_prev()

                # ---- solo loop: 2 sk tiles per step on alternating halves ----
                def s_solo_start(st):
                    i0, i1 = 2 * st, 2 * st + 1
                    c0, c1 = col0_of(i0), col0_of(i1)
                    s_ps = psum.tile([P, 2, SQT], fp32, tag="s", name="s_ps3",
                                     bufs=2)
                    nc.tensor.matmul(
                        s_ps[:, 0, c0:SQT],
                        KT3d[0:64, ts(i0, P)],
                        QT3d[0:64, ds(SQT * j + c0, SQT - c0)],
                        start=True, stop=True)
                    nc.tensor.matmul(
                        s_ps[:, 1, c1:SQT],
                        KT3d[64:128, ts(i1, P)],
                        QT3d[64:128, ds(SQT * j + c1, SQT - c1)],
                        start=True, stop=True)
                    return s_ps

                n_st = n_i // 2
                def norm_full(h, last=last):
                    def f():
                        norm_b(h, z_pss[h],
                               norm_a(z_pss[h],
                                      bcast=("mm" if last else "dram")))
                    return f

                local_work = [norm_full(0), norm_full(1)]
                pend = s_solo_start(0)
                for st in range(n_st):
                    i0, i1 = 2 * st, 2 * st + 1
                    c0, c1 = col0_of(i0), col0_of(i1)
                    s_ps = pend
                    E_t = epool.tile([P, 2, SQT], bf16, tag="ep3", name="E_t3",
                                     bufs=3)
                    if c0 == c1:
                        nc.scalar.activation(E_t[:, :, c0:SQT],
                                             s_ps[:, :, c0:SQT], Exp,
                                             scale=0.125)
                    else:
                        nc.scalar.activation(E_t[:, 0, c0:SQT],
                                             s_ps[:, 0, c0:SQT], Exp,
                                             scale=0.125)
                        nc.scalar.activation(E_t[:, 1, c1:SQT],
                                             s_ps[:, 1, c1:SQT], Exp,
                                             scale=0.125)
                    if i0 >= 4 * j:
                        nc.vector.tensor_tensor(
                            E_t[:, 0, c0:c0 + P], E_t[:, 0, c0:c0 + P],
                            maskb[:], Mult)
                    if i1 >= 4 * j:
                        nc.vector.tensor_tensor(
                            E_t[:, 1, c1:c1 + P], E_t[:, 1, c1:c1 + P],
                            maskb[:], Mult)
                    if st + 1 < n_st:
                        pend = s_solo_start(st + 1)
                    if local_work:
                        local_work.pop(0)()
                    else:
                        pop_prev()
                    nc.tensor.matmul(
                        z_pss[2][0:65, c0:SQT], V_all[:, i0, 2, :],
                        E_t[:, 0, c0:SQT],
                        start=(i0 == 0), stop=False, skip_group_check=True)
                    nc.tensor.matmul(
                        z_pss[2][0:65, c1:SQT], V_all[:, i1, 2, :],
                        E_t[:, 1, c1:SQT],
                        start=False, stop=(i1 == n_i - 1),
                        skip_group_check=True)
                    pop_prev()

                for f in local_work:
                    f()
                rbc2 = norm_a(z_pss[2], bcast=("mm" if last else "dram"))

                # ---- next column's q2/k2 so its scores start at once;
                # qk3 + V ride the next column's pop queue ----
                nextproj = []
                if j + 1 < NJ:
                    qk_group(j + 1, 0, "s")
                    qk_group(j + 1, 1, "s")
                    nextproj = [lambda jj=j + 1: qk_group(jj, 2, "o")]
                    nextproj += [lambda t=t: v_group(t, "o")
                                 for t in range(4 * j + 4, 4 * j + 8)]
                else:
                    # keep the PE warm across the final normalize chain so
                    # the out-projection runs at full clock
                    fill_big(10)
                norm_b(2, z_pss[2], rbc2)

                # ---- out-proj closures for this column ----
                otag = "s" if last else "o"

                def make_closures(j, zT2, zT3, otag):
                    clos = []
                    for c in range(4):
                        def f1(c=c, zT2=zT2, zT3=zT3):
                            row = ds(SQT * j + P * c, P)
                            os_t = zsb.tile([P, D], bf16, tag="os", name="os",
                                            bufs=3)
                            o1 = psum.tile([P, 512], fp32, tag=otag, name="o1",
                                           bufs=(2 if otag == "s" else 1))
                            nc.tensor.matmul(o1[:], zT2[:, ts(c, P)],
                                             wo2b[:, 0:512],
                                             start=True, stop=False,
                                             skip_group_check=True)
                            nc.tensor.matmul(o1[:], zT3[:, ts(c, P)],
                                             wo3b[:, 0:512],
                                             start=False, stop=True,
                                             skip_group_check=True)
                            nc.vector.tensor_copy(os_t[:, 0:512], o1[:])
                            return os_t
                        def f2(c=c, zT2=zT2, zT3=zT3, f1=f1):
                            os_t = f1.os_t
                            row = ds(SQT * j + P * c, P)
                            o2 = psum.tile([P, 512], fp32, tag=otag, name="o2",
                                           bufs=(2 if otag == "s" else 1))
                            nc.tensor.matmul(o2[:, 0:256], zT2[:, ts(c, P)],
                                             wo2b[:, 512:768],
                                             start=True, stop=False,
                                             skip_group_check=True)
                            nc.tensor.matmul(o2[:, 0:256], zT3[:, ts(c, P)],
                                             wo3b[:, 512:768],
                                             start=False, stop=True,
                                             skip_group_check=True)
                            nc.vector.tensor_copy(os_t[:, 512:768],
                                                  o2[:, 0:256])
                            nc.sync.dma_start(out[row, :], os_t[:])
                        def g1(f1=f1):
                            f1.os_t = f1()
                        clos.append(g1)
                        clos.append(f2)
                    return clos

                for f in prev_work:
                    f()
                prev_work = nextproj + make_closures(j, zT2, zT3, otag)
                if last:
                    for f in prev_work:
                        f()
                        fill_mm(2)
                    prev_work = []

    nc.compile()
    return nc


def _get_program():
    global _PROGRAM
    if _PROGRAM is None:
        _PROGRAM = _build_program()
    return _PROGRAM


def kernel(x, W_Q, W_K, W_V, W_O, b_Q, b_K, b_V, b_O):
    global LAST_RESULTS
    _install_ntff_shim()
    import ml_dtypes
    from concourse import bass_utils

    bf = ml_dtypes.bfloat16
    x = np.asarray(x, dtype=np.float32)
    W_Q = np.asarray(W_Q, dtype=np.float32)
    W_K = np.asarray(W_K, dtype=np.float32)
    W_V = np.asarray(W_V, dtype=np.float32)
    W_O = np.asarray(W_O, dtype=np.float32)
    b_Q = np.asarray(b_Q, dtype=np.float32)
    b_K = np.asarray(b_K, dtype=np.float32)
    b_V = np.asarray(b_V, dtype=np.float32)
    b_O = np.asarray(b_O, dtype=np.float32)
    assert not (np.any(b_Q) or np.any(b_K) or np.any(b_V)), \
        "kernel assumes zero QKV biases (problem spec fill=zeros)"

    nc = _get_program()

    mask = np.triu(np.ones((P, P), dtype=np.float32)).astype(bf)
    xTs = [np.ascontiguousarray(x[b].T).astype(bf) for b in range(B)]

    def pack(w):
        # [D, E] -> [p=128, o=6, E] matching the SBUF tile layout
        return np.ascontiguousarray(
            w.reshape(NK, P, w.shape[1]).transpose(1, 0, 2)).astype(bf)

    in_maps = []
    for c in range(N_CORES):
        b, g = c // 4, c % 4
        hs = [3 * g, 3 * g + 1, 3 * g + 2]
        in_maps.append({
            "xT": xTs[b],
            "wq2": pack(np.concatenate([W_Q[hs[0]], W_Q[hs[1]]], axis=1)),
            "wk2": pack(np.concatenate([W_K[hs[0]], W_K[hs[1]]], axis=1)),
            "wqk3": pack(np.concatenate([W_Q[hs[2]], W_K[hs[2]]], axis=1)),
            "wv": pack(np.concatenate(
                [W_V[hs[0]], W_V[hs[1]], W_V[hs[2]]], axis=1)),
            "wo2": np.ascontiguousarray(
                np.concatenate([W_O[hs[0]], W_O[hs[1]]], axis=0)).astype(bf),
            "wo3": np.ascontiguousarray(W_O[hs[2]]).astype(bf),
            "mask": mask,
        })

    res = bass_utils.run_bass_kernel_spmd(
        nc, in_maps, core_ids=list(range(N_CORES)),
        trace=bool(os.environ.get("BASS_TRACE")))
    LAST_RESULTS = res

    parts = [res.results[c]["out"].astype(np.float32) for c in range(N_CORES)]
    full = np.stack([
        parts[0] + parts[1] + parts[2] + parts[3],
        parts[4] + parts[5] + parts[6] + parts[7],
    ], axis=0)
    if np.any(b_O):
        full = full + b_O
    return full.astype(np.float32)


# revision 28
# speedup vs baseline: 1.0319x; 1.0319x over previous
"""Causal multi-head attention block on 8 trn2 NeuronCores.

Problem: B=2, S=2048, D=768, H=12, Dh=64 (fp32), causal softmax attention
with QKV projections and output projection summed over heads.

Sharding: tensor-parallel over heads x data-parallel over batch.
core c in [0,8): b = c//4, heads = {3g, 3g+1, 3g+2} with g = c%4.
Each core computes the partial output sum over its 3 heads for its batch;
the host sums the 4 partials per batch (the TP all-reduce) and stacks.

v5 layout (all device I/O in bf16, host pre-casts/pre-packs):
  - x^T [768, 2048] bf16 loaded directly to SBUF (no on-device cast).
  - QKV projections per sq-column j are emitted interleaved with the
    attention loops of column j-1 so ScalarE exp work starts early.
  - Pair heads (h0,h1) stacked on PE row halves -> concurrent K=64 score
    matmuls; solo head h2 duplicated on both partition halves and two
    sk-tiles processed per step on alternating halves (also concurrent),
    with a single merged exp per step.
  - z matmuls carry a ones column for the softmax denominator; normalize
    uses reciprocal straight from PSUM + row broadcast + one fp32 multiply.
  - Output projection accumulates the head pair (K=128) and solo (K=64)
    into shared PSUM, copied to bf16 and stored as one DMA per row block.

Biases are all zeros per the problem spec (fill=zeros); b_O is applied on
the host if nonzero (exact). b_Q/b_K/b_V are asserted zero.
"""

import os
import sys
import types
import numpy as np

B, S, D, H, DH = 2, 2048, 768, 12, 64
N_CORES = 8
P = 128
NK = D // P      # 6 contraction chunks
NJ = S // 512    # 4 sq tiles of 512
NI = S // P      # 16 sk tiles of 128
SQT = 512

# SBUF->SBUF broadcast DMA for the denominator row; if the toolchain
# rejects it we fall back to a DRAM bounce.
SBUF_BCAST = os.environ.get("BASS_SBUF_BCAST", "0") == "1"

_PROGRAM = None
LAST_RESULTS = None


def _install_ntff_shim():
    """antenv.axon_hooks is missing in this image; shim it so trace=True works."""
    if "antenv.axon_hooks" in sys.modules:
        return
    try:
        from trn_agent_boot.trn_boot import _ntff_profile_via_ctypes
        m = types.ModuleType("antenv.axon_hooks")
        hook = _ntff_profile_via_ctypes("/opt/axon/libaxon_pjrt.so")
        m.get_axon_ntff_profile_hook = lambda: hook
        m.set_axon_ntff_profile_hook = lambda h: None
        sys.modules["antenv.axon_hooks"] = m
    except Exception:
        pass


def _build_program():
    import concourse.bass as bass
    import concourse.mybir as mybir
    from concourse import bacc
    from concourse.tile import TileContext
    from concourse.bass import ts, ds

    fp32 = mybir.dt.float32
    bf16 = mybir.dt.bfloat16
    Exp = mybir.ActivationFunctionType.Exp
    Mult = mybir.AluOpType.mult

    nc = bacc.Bacc("TRN2", target_bir_lowering=False, debug=False,
                   num_devices=N_CORES)

    xT = nc.dram_tensor("xT", (D, S), bf16, kind="ExternalInput")
    wq2 = nc.dram_tensor("wq2", (P, NK, 128), bf16, kind="ExternalInput")
    wk2 = nc.dram_tensor("wk2", (P, NK, 128), bf16, kind="ExternalInput")
    wqk3 = nc.dram_tensor("wqk3", (P, NK, 128), bf16, kind="ExternalInput")
    wvb_src = nc.dram_tensor("wv", (P, NK, 192), bf16, kind="ExternalInput")
    wo2 = nc.dram_tensor("wo2", (128, D), bf16, kind="ExternalInput")
    wo3 = nc.dram_tensor("wo3", (DH, D), bf16, kind="ExternalInput")
    maskin = nc.dram_tensor("mask", (P, P), bf16, kind="ExternalInput")
    out = nc.dram_tensor("out", (S, D), bf16, kind="ExternalOutput")

    with TileContext(nc) as tc:
        with tc.tile_pool(name="work", bufs=1) as work, \
             tc.tile_pool(name="epool", bufs=4) as epool, \
             tc.tile_pool(name="zsb", bufs=2) as zsb, \
             tc.tile_pool(name="zcol", bufs=2) as zcol, \
             tc.tile_pool(name="dram", bufs=2, space="DRAM") as dram, \
             tc.tile_pool(name="psum", bufs=2, space="PSUM") as psum:

            # ---------------- persistent SBUF tiles ----------------
            QT2 = work.tile([P, S], bf16, name="QT2")   # h0 rows 0:64, h1 rows 64:128
            KT2 = work.tile([P, S], bf16, name="KT2")
            QT3d = work.tile([P, S], bf16, name="QT3d")  # h2 Q^T duplicated both halves
            KT3d = work.tile([P, S], bf16, name="KT3d")  # h2 K^T duplicated both halves
            V_all = work.tile([P, NI, 3, 65], bf16, name="V_all")
            xTb = work.tile([P, NK, S], bf16, name="xTb")
            wq2b = work.tile([P, NK, 128], bf16, name="wq2b")
            wk2b = work.tile([P, NK, 128], bf16, name="wk2b")
            wqk3b = work.tile([P, NK, 128], bf16, name="wqk3b")
            wvb = work.tile([P, NK, 192], bf16, name="wvb")
            wo2b = work.tile([P, D], bf16, name="wo2b")
            wo3b = work.tile([64, D], bf16, name="wo3b")
            maskb = work.tile([P, P], bf16, name="maskb")
            warm = work.tile([1, 8], fp32, name="warm")

            # ---------------- input DMAs (no casts needed) ----------------
            # d-major full-row x chunks (fat descriptors), split across both
            # HWDGE rings (sync + scalar); weights needed first lead each ring.
            nc.sync.dma_start(wq2b[:], wq2[:])
            for k in range(NK):
                eng = nc.sync if (k % 2 == 0) else nc.scalar
                eng.dma_start(xTb[:, k, 0:1024], xT[ts(k, P), 0:1024])
            nc.sync.dma_start(wk2b[:], wk2[:])
            nc.sync.dma_start(wqk3b[:], wqk3[:])
            nc.scalar.dma_start(wvb[:], wvb_src[:])
            for k in range(NK):
                eng = nc.sync if (k % 2 == 0) else nc.scalar
                eng.dma_start(xTb[:, k, 1024:2048], xT[ts(k, P), 1024:2048])
            nc.scalar.dma_start(wo2b[:], wo2[:])
            nc.scalar.dma_start(wo3b[:], wo3[:])
            nc.scalar.dma_start(maskb[:], maskin[:])

            # Pre-load the exp activation table while DMAs land.
            nc.vector.memset(warm[:], 0.0)
            nc.scalar.activation(warm[:], warm[:], Exp)
            nc.vector.memset(V_all[:, :, :, 64], 1.0)

            # ---------------- phase emitters ----------------

            wpfill = psum.tile([64, 64], fp32, tag="o", name="wpfill",
                               bufs=1)

            wpf2 = psum.tile([64, SQT], fp32, tag="o", name="wpf2", bufs=1)

            def fill_mm(n):
                for _ in range(n):
                    nc.tensor.matmul(wpfill[:], wdum[:, 0:64], wdum[:, 64:128],
                                     start=True, stop=True,
                                     skip_group_check=True)

            def fill_big(n):
                for _ in range(n):
                    nc.tensor.matmul(wpf2[:], wdum[:, 0:64], wdumw[:],
                                     start=True, stop=True,
                                     skip_group_check=True)

            def qk_group(st_i, which, tag, fills=0):
                sl = ts(st_i, SQT)
                pt = psum.tile([P, SQT], fp32, tag=tag, name=f"qkp{which}",
                               bufs=(2 if tag == "s" else 1))
                wsel = (wq2b, wk2b, wqk3b)[which]
                for k in range(NK):
                    if fills:
                        fill_mm(fills)
                    nc.tensor.matmul(pt[:], wsel[:, k, :], xTb[:, k, sl],
                                     start=(k == 0), stop=(k == NK - 1))
                if which == 0:
                    nc.vector.tensor_copy(QT2[:, sl], pt[:])
                elif which == 1:
                    nc.vector.tensor_copy(KT2[:, sl], pt[:])
                else:
                    nc.vector.tensor_copy(QT3d[0:64, sl], pt[0:64, :])
                    nc.vector.tensor_copy(KT3d[64:128, sl], pt[64:128, :])
                    # duplicate to the other half for row-tiled matmuls
                    nc.sync.dma_start(QT3d[64:128, sl], QT3d[0:64, sl])
                    nc.sync.dma_start(KT3d[0:64, sl], KT3d[64:128, sl])

            def v_group(t, tag):
                vp = psum.tile([P, SQT], fp32, tag=tag, name="vp",
                               bufs=(2 if tag == "s" else 1))
                for k in range(NK):
                    nc.tensor.matmul(vp[:, 0:192], xTb[:, k, ts(t, P)],
                                     wvb[:, k, :],
                                     start=(k == 0), stop=(k == NK - 1))
                nc.vector.tensor_copy(
                    V_all[:, t, :, 0:64],
                    vp[:, 0:192].rearrange("p (h e) -> p h e", h=3))

            wdum = work.tile([64, P], bf16, name="wdum")
            wdumw = work.tile([64, SQT], bf16, name="wdumw")
            ones64 = work.tile([1, 64], bf16, name="ones64")
            nc.vector.memset(wdum[:], 0.0)
            nc.vector.memset(wdumw[:], 0.0)
            nc.vector.memset(ones64[:], 1.0)
            fill_big(14)
            qk_group(0, 0, "s", fills=2)
            qk_group(0, 1, "s")
            qk_group(0, 2, "s")

            # ---------------- attention ----------------

            prev_work = []  # deferred out-proj closures from the previous column

            def pop_prev():
                if prev_work:
                    prev_work.pop(0)()

            for j in range(NJ):
                zT2 = zcol.tile([P, SQT], bf16, tag="zT2", name="zT2")
                zT3 = zcol.tile([64, SQT], bf16, tag="zT3", name="zT3")
                n_i = 4 * j + 4
                last = (j == NJ - 1)

                def col0_of(i):
                    return P * (i - 4 * j) if i >= 4 * j else 0

                z_pss = [psum.tile([P, SQT], fp32, tag="z", name=f"z_ps{h}",
                                   bufs=3) for h in range(3)]

                def norm_a(z_ps, bcast="dram"):
                    # tensor_copy handles the partition-64 -> 0 shift; the
                    # custom-DVE reciprocal does not (reads the wrong lane).
                    drow = zsb.tile([1, SQT], fp32, tag="drow", name="drow",
                                    bufs=3)
                    nc.vector.tensor_copy(drow[:], z_ps[64:65, :])
                    rsb = zsb.tile([1, SQT], fp32, tag="rsb", name="rsb", bufs=3)
                    nc.vector.reciprocal_approx_fast(rsb[:], drow[:])
                    rbc = zsb.tile([64, SQT], fp32, tag="rbc", name="rbc", bufs=3)
                    if bcast == "mm":
                        rs16 = zsb.tile([1, SQT], bf16, tag="rs16", name="rs16",
                                        bufs=2)
                        nc.vector.tensor_copy(rs16[:], rsb[:])
                        obp = psum.tile([64, SQT], fp32, tag="o", name="obp",
                                        bufs=1)
                        nc.tensor.matmul(obp[:], ones64[:], rs16[:],
                                         start=True, stop=True,
                                         skip_group_check=True)
                        nc.vector.tensor_copy(rbc[:], obp[:])
                    else:
                        rscr = dram.tile([1, SQT], fp32, name="rscr")
                        nc.sync.dma_start(rscr[:], rsb[:])
                        nc.sync.dma_start(rbc[:], rscr[:].to_broadcast((64, SQT)))
                    return rbc

                def norm_b(h, z_ps, rbc, zT2=zT2, zT3=zT3):
                    if h == 0:
                        nc.vector.tensor_tensor(zT2[0:64, :], z_ps[0:64, :],
                                                rbc[:], Mult)
                    elif h == 1:
                        z1t = zcol.tile([64, SQT], bf16, tag="z1t", name="z1t")
                        nc.vector.tensor_tensor(z1t[:], z_ps[0:64, :],
                                                rbc[:], Mult)
                        nc.sync.dma_start(zT2[64:128, :], z1t[:])
                    else:
                        nc.vector.tensor_tensor(zT3[:], z_ps[0:64, :],
                                                rbc[:], Mult)

                # ---- pair i-loop ----
                def s_pair_start(i):
                    col0 = col0_of(i)
                    s_ps = psum.tile([P, 2, SQT], fp32, tag="s", name="s_ps",
                                     bufs=2)
                    for h in range(2):
                        nc.tensor.matmul(
                            s_ps[:, h, col0:SQT],
                            KT2[64 * h:64 * h + 64, ts(i, P)],
                            QT2[64 * h:64 * h + 64, ds(SQT * j + col0,
                                                       SQT - col0)],
                            start=True, stop=True)
                    return s_ps, col0

                pend = s_pair_start(0)
                if j == 0:
                    # emitted after the first score matmuls so the first exp
                    # issues as early as possible
                    for t in range(4):
                        v_group(t, "s")
                for i in range(n_i):
                    s_ps, col0 = pend
                    E_t = epool.tile([P, 2, SQT], bf16, tag="ep", name="E_t",
                                     bufs=4)
                    nc.scalar.activation(E_t[:, :, col0:SQT],
                                         s_ps[:, :, col0:SQT], Exp, scale=0.125)
                    if i >= 4 * j:
                        nc.vector.tensor_tensor(
                            E_t[:, :, col0:col0 + P], E_t[:, :, col0:col0 + P],
                            maskb[:, None, :].to_broadcast((P, 2, P)), Mult)
                    if i + 1 < n_i:
                        pend = s_pair_start(i + 1)
                    pop_prev()
                    for h in range(2):
                        nc.tensor.matmul(
                            z_pss[h][0:65, col0:SQT],
                            V_all[:, i, h, :],
                            E_t[:, h, col0:SQT],
                            start=(i == 0), stop=(i == n_i - 1),
                            skip_group_check=True)
                    pop_prev()

                # ---- solo loop: 2 sk tiles per step on alternating halves ----
                def s_solo_start(st):
                    i0, i1 = 2 * st, 2 * st + 1
                    c0, c1 = col0_of(i0), col0_of(i1)
                    s_ps = psum.tile([P, 2, SQT], fp32, tag="s", name="s_ps3",
                                     bufs=2)
                    nc.tensor.matmul(
                        s_ps[:, 0, c0:SQT],
                        KT3d[0:64, ts(i0, P)],
                        QT3d[0:64, ds(SQT * j + c0, SQT - c0)],
                        start=True, stop=True)
                    nc.tensor.matmul(
                        s_ps[:, 1, c1:SQT],
                        KT3d[64:128, ts(i1, P)],
                        QT3d[64:128, ds(SQT * j + c1, SQT - c1)],
                        start=True, stop=True)
                    return s_ps

                n_st = n_i // 2
                def norm_full(h):
                    def f():
                        norm_b(h, z_pss[h], norm_a(z_pss[h]))
                    return f

                if n_st >= 4:
                    celln = {}

                    def a_of(h):
                        def f():
                            celln[h] = norm_a(z_pss[h])
                        return f

                    def b_of(h):
                        def f():
                            norm_b(h, z_pss[h], celln[h])
                        return f

                    local_work = [a_of(0), a_of(1), b_of(0), b_of(1)]
                else:
                    local_work = [norm_full(0), norm_full(1)]
                pend = s_solo_start(0)
                for st in range(n_st):
                    i0, i1 = 2 * st, 2 * st + 1
                    c0, c1 = col0_of(i0), col0_of(i1)
                    s_ps = pend
                    E_t = epool.tile([P, 2, SQT], bf16, tag="ep3", name="E_t3",
                                     bufs=3)
                    if c0 == c1:
                        nc.scalar.activation(E_t[:, :, c0:SQT],
                                             s_ps[:, :, c0:SQT], Exp,
                                             scale=0.125)
                    else:
                        nc.scalar.activation(E_t[:, 0, c0:SQT],
                                             s_ps[:, 0, c0:SQT], Exp,
                                             scale=0.125)
                        nc.scalar.activation(E_t[:, 1, c1:SQT],
                                             s_ps[:, 1, c1:SQT], Exp,
                                             scale=0.125)
                    if i0 >= 4 * j:
                        nc.vector.tensor_tensor(
                            E_t[:, 0, c0:c0 + P], E_t[:, 0, c0:c0 + P],
                            maskb[:], Mult)
                    if i1 >= 4 * j:
                        nc.vector.tensor_tensor(
                            E_t[:, 1, c1:c1 + P], E_t[:, 1, c1:c1 + P],
                            maskb[:], Mult)
                    if st + 1 < n_st:
                        pend = s_solo_start(st + 1)
                    if local_work:
                        local_work.pop(0)()
                    else:
                        pop_prev()
                    nc.tensor.matmul(
                        z_pss[2][0:65, c0:SQT], V_all[:, i0, 2, :],
                        E_t[:, 0, c0:SQT],
                        start=(i0 == 0), stop=False, skip_group_check=True)
                    nc.tensor.matmul(
                        z_pss[2][0:65, c1:SQT], V_all[:, i1, 2, :],
                        E_t[:, 1, c1:SQT],
                        start=False, stop=(i1 == n_i - 1),
                        skip_group_check=True)
                    pop_prev()

                for f in local_work:
                    f()
                rbc2 = norm_a(z_pss[2], bcast=("mm" if last else "dram"))

                # ---- next column's q2/k2 so its scores start at once;
                # qk3 + V ride the next column's pop queue ----
                nextproj = []
                if j + 1 < NJ:
                    qk_group(j + 1, 0, "s")
                    qk_group(j + 1, 1, "s")
                    nextproj = [lambda jj=j + 1: qk_group(jj, 2, "o")]
                    nextproj += [lambda t=t: v_group(t, "o")
                                 for t in range(4 * j + 4, 4 * j + 8)]
                else:
                    # keep the PE warm across the final normalize chain so
                    # the out-projection runs at full clock
                    fill_mm(16)
                norm_b(2, z_pss[2], rbc2)

                # ---- out-proj closures for this column ----
                otag = "s" if last else "o"

                def make_closures(j, zT2, zT3, otag):
                    clos = []
                    for c in range(4):
                        def f1(c=c, zT2=zT2, zT3=zT3):
                            row = ds(SQT * j + P * c, P)
                            os_t = zsb.tile([P, D], bf16, tag="os", name="os",
                                            bufs=3)
                            o1 = psum.tile([P, 512], fp32, tag=otag, name="o1",
                                           bufs=(2 if otag == "s" else 1))
                            nc.tensor.matmul(o1[:], zT2[:, ts(c, P)],
                                             wo2b[:, 0:512],
                                             start=True, stop=False,
                                             skip_group_check=True)
                            nc.tensor.matmul(o1[:], zT3[:, ts(c, P)],
                                             wo3b[:, 0:512],
                                             start=False, stop=True,
                                             skip_group_check=True)
                            nc.vector.tensor_copy(os_t[:, 0:512], o1[:])
                            return os_t
                        def f2(c=c, zT2=zT2, zT3=zT3, f1=f1):
                            os_t = f1.os_t
                            row = ds(SQT * j + P * c, P)
                            o2 = psum.tile([P, 512], fp32, tag=otag, name="o2",
                                           bufs=(2 if otag == "s" else 1))
                            nc.tensor.matmul(o2[:, 0:256], zT2[:, ts(c, P)],
                                             wo2b[:, 512:768],
                                             start=True, stop=False,
                                             skip_group_check=True)
                            nc.tensor.matmul(o2[:, 0:256], zT3[:, ts(c, P)],
                                             wo3b[:, 512:768],
                                             start=False, stop=True,
                                             skip_group_check=True)
                            nc.vector.tensor_copy(os_t[:, 512:768],
                                                  o2[:, 0:256])
                            nc.sync.dma_start(out[row, :], os_t[:])
                        def g1(f1=f1):
                            f1.os_t = f1()
                        clos.append(g1)
                        clos.append(f2)
                    return clos

                for f in prev_work:
                    f()
                prev_work = nextproj + make_closures(j, zT2, zT3, otag)
                if last:
                    for f in prev_work:
                        f()
                        fill_mm(2)
                    prev_work = []

    nc.compile()
    return nc


def _get_program():
    global _PROGRAM
    if _PROGRAM is None:
        _PROGRAM = _build_program()
    return _PROGRAM


def kernel(x, W_Q, W_K, W_V, W_O, b_Q, b_K, b_V, b_O):
    global LAST_RESULTS
    _install_ntff_shim()
    import ml_dtypes
    from concourse import bass_utils

    bf = ml_dtypes.bfloat16
    x = np.asarray(x, dtype=np.float32)
    W_Q = np.asarray(W_Q, dtype=np.float32)
    W_K = np.asarray(W_K, dtype=np.float32)
    W_V = np.asarray(W_V, dtype=np.float32)
    W_O = np.asarray(W_O, dtype=np.float32)
    b_Q = np.asarray(b_Q, dtype=np.float32)
    b_K = np.asarray(b_K, dtype=np.float32)
    b_V = np.asarray(b_V, dtype=np.float32)
    b_O = np.asarray(b_O, dtype=np.float32)
    assert not (np.any(b_Q) or np.any(b_K) or np.any(b_V)), \
        "kernel assumes zero QKV biases (problem spec fill=zeros)"

    nc = _get_program()

    mask = np.triu(np.ones((P, P), dtype=np.float32)).astype(bf)
    xTs = [np.ascontiguousarray(x[b].T).astype(bf) for b in range(B)]

    def pack(w):
        # [D, E] -> [p=128, o=6, E] matching the SBUF tile layout
        return np.ascontiguousarray(
            w.reshape(NK, P, w.shape[1]).transpose(1, 0, 2)).astype(bf)

    in_maps = []
    for c in range(N_CORES):
        b, g = c // 4, c % 4
        hs = [3 * g, 3 * g + 1, 3 * g + 2]
        in_maps.append({
            "xT": xTs[b],
            "wq2": pack(np.concatenate([W_Q[hs[0]], W_Q[hs[1]]], axis=1)),
            "wk2": pack(np.concatenate([W_K[hs[0]], W_K[hs[1]]], axis=1)),
            "wqk3": pack(np.concatenate([W_Q[hs[2]], W_K[hs[2]]], axis=1)),
            "wv": pack(np.concatenate(
                [W_V[hs[0]], W_V[hs[1]], W_V[hs[2]]], axis=1)),
            "wo2": np.ascontiguousarray(
                np.concatenate([W_O[hs[0]], W_O[hs[1]]], axis=0)).astype(bf),
            "wo3": np.ascontiguousarray(W_O[hs[2]]).astype(bf),
            "mask": mask,
        })

    res = bass_utils.run_bass_kernel_spmd(
        nc, in_maps, core_ids=list(range(N_CORES)),
        trace=bool(os.environ.get("BASS_TRACE")))
    LAST_RESULTS = res

    parts = [res.results[c]["out"].astype(np.float32) for c in range(N_CORES)]
    full = np.stack([
        parts[0] + parts[1] + parts[2] + parts[3],
        parts[4] + parts[5] + parts[6] + parts[7],
    ], axis=0)
    if np.any(b_O):
        full = full + b_O
    return full.astype(np.float32)
